# revision 23
# baseline (speedup 1.0000x reference)
"""Trainium2 Bass kernel for cross-attention:
    scores  = dec @ enc^T            [B, Tq, Tk]
    probs   = softmax(scores, -1)
    context = probs @ enc            [B, Tq, D]

Shapes (hardcoded): enc [16, 2048, 1024] f32, dec [16, 128, 1024] f32.
Sharding: data-parallel over batch B across 8 NeuronCores (2 batches/core).

Per-core flow, software-pipelined over 8 global chunks (2 batches x 4
chunks of 512 k-rows):
  stage A(g): DMA chunk; transpose-mode matmuls -> encT [d, k] tiles
              (PSUM -> SBUF copies split DVE/ACT); mm1 (fp32r, N=512)
              accumulates scores [q, 512]; ACT exp streams the chunk
              (softmax shift = chunk-0 row max; exact, shift-invariant)
              with accum_out partial sums.
  stage B(g): probs chunk PE-transposed -> probsT [k, q]; mm2 (fp32r)
              accumulates context from the resident natural enc tiles.
  B(g-1) is emitted after A(g) so the PE never head-of-line blocks on
  the exp -> probsT dependency.
Output stores ride SWDGE (gpsimd) so the Sync DMA queue streams input
chunks back-to-back. A dummy-matmul burst at kernel start (hidden
under the first DMA) flips the PE HAM clock gate to 2.4 GHz early.
fp32r keeps ~13 effective mantissa bits (measured ~1.5e-4).
"""

import sys

sys.path.insert(0, "/opt/trn_rl_repo")

import numpy as np
from contextlib import ExitStack

import concourse.bass as bass
import concourse.tile as tile
from concourse import bacc, mybir
from concourse.masks import make_identity

F32 = mybir.dt.float32
F32R = mybir.dt.float32r
BF16 = mybir.dt.bfloat16
EXP = mybir.ActivationFunctionType.Exp
COPY = mybir.ActivationFunctionType.Copy
AX_X = mybir.AxisListType.X

B, Tk, Tq, D = 16, 2048, 128, 1024
CORES = 8
BLOC = B // CORES          # batches per core
KCH = 4                    # k chunks per batch
KCS = Tk // KCH            # 512 k rows per chunk
NSUB = KCS // 128          # 4 k-subtiles per chunk
DT = D // 128              # 8 d-tiles
DH = D // 512              # 2 output column halves
NCHUNK = BLOC * KCH        # global pipeline length
NWARM = 10                 # dummy matmuls to warm the HAM clock gate

_CACHE = {}


def _build():
    nc = bacc.Bacc("TRN2", debug=False, num_devices=CORES)
    enc = nc.dram_tensor("enc", [BLOC, Tk, D], F32, kind="ExternalInput").ap()
    dec = nc.dram_tensor("dec", [BLOC, Tq, D], F32, kind="ExternalInput").ap()
    out = nc.dram_tensor("out", [BLOC, Tq, D], F32, kind="ExternalOutput").ap()

    with tile.TileContext(nc) as tc, ExitStack() as ctx:
        sb = ctx.enter_context(tc.tile_pool(name="sb", bufs=1))
        enc_p = ctx.enter_context(tc.tile_pool(name="enc", bufs=7))
        encT_p = ctx.enter_context(tc.tile_pool(name="encT", bufs=10))
        dec_p = ctx.enter_context(tc.tile_pool(name="dec", bufs=2))
        decT_p = ctx.enter_context(tc.tile_pool(name="decT", bufs=4))
        probs_p = ctx.enter_context(tc.tile_pool(name="probs", bufs=1))
        probsT_p = ctx.enter_context(tc.tile_pool(name="probsT", bufs=8))
        scsb_p = ctx.enter_context(tc.tile_pool(name="scsb", bufs=4))
        outp_p = ctx.enter_context(tc.tile_pool(name="outp", bufs=2))
        stat_p = ctx.enter_context(tc.tile_pool(name="stat", bufs=4))
        sc_p = ctx.enter_context(tc.tile_pool(name="sc", bufs=2, space="PSUM"))
        tr_p = ctx.enter_context(tc.tile_pool(name="tr", bufs=4, space="PSUM"))
        ctx_p = ctx.enter_context(tc.tile_pool(name="ctx", bufs=2, space="PSUM"))

        # HAM warm-up: independent of the identity-creation chain; only
        # needs the memset. Runs back-to-back while the first enc chunk
        # DMA streams in. fp32 N=128 is 4 cyc/row -> ~0.4us each cold.
        # operand data is irrelevant (the result is never consumed);
        # memset is the fastest producer after the entry barrier.
        junk = sb.tile([128, 128], F32)
        nc.gpsimd.memset(junk[:], 0.0)
        warm = sc_p.tile([128, 512], F32, tag="sc", name="warm")
        for i in range(NWARM):
            nc.tensor.matmul(
                warm[:, 0:128], junk[:], junk[:],
                start=(i == 0), stop=(i == NWARM - 1),
            )

        ident = sb.tile([128, 128], F32)
        ident_r = sb.tile([128, 128], F32R)
        make_identity(nc, ident[:])
        nc.vector.tensor_copy(ident_r[:], ident[:])

        copy_count = [0]

        def psum2sbuf(dst, src):
            if copy_count[0] % 2 == 0:
                nc.vector.tensor_copy(dst, src)
            else:
                nc.scalar.copy(dst, src)
            copy_count[0] += 1

        # per-batch state
        state = {}

        def begin_batch(b):
            st = {}
            st["negmax"] = stat_p.tile([128, 1], F32, tag="negmax", name=f"nm{b}")
            st["negsh"] = stat_p.tile([128, 1], F32, tag="negsh", name=f"ns{b}")
            st["probs"] = probs_p.tile([128, Tk], F32, tag="probs", name=f"pr{b}")
            st["sums"] = stat_p.tile([128, KCH], F32, tag="sums", name=f"sm{b}")
            st["cps"] = [
                ctx_p.tile([128, 512], F32, tag="ctx", name=f"cps{b}_{dh}")
                for dh in range(DH)
            ]
            st["enc_sb"] = {}
            # dec load + transposes -> decT [d, q]
            dec_sb = dec_p.tile([128, D], F32R, tag="dec", name=f"dec{b}")
            nc.sync.dma_start(dec_sb[:], dec[b].bitcast(F32R))
            decT = []
            for blk in range(2):
                trt = tr_p.tile([128, 512], F32R, tag="tr", name=f"trd{b}_{blk}")
                for j in range(4):
                    dd = 4 * blk + j
                    nc.tensor.transpose(
                        trt[:, 128 * j : 128 * (j + 1)],
                        dec_sb[:, 128 * dd : 128 * (dd + 1)],
                        ident_r[:],
                    )
                dstT = decT_p.tile([128, 512], F32R, tag="decT", name=f"dT{b}_{blk}")
                psum2sbuf(dstT[:], trt[:])
                decT.append(dstT)
            st["decT"] = decT
            state[b] = st

        def stage_a(g):
            b, kc = divmod(g, KCH)
            et = enc_p.tile([128, NSUB, D], F32R, tag="enc", name=f"enc{g}")
            nc.sync.dma_start(
                et[:],
                enc[b, kc * KCS : (kc + 1) * KCS, :]
                .rearrange("(n p) d -> p n d", p=128)
                .bitcast(F32R),
            )
            if kc == 0:
                begin_batch(b)
            st = state[b]
            st["enc_sb"][kc] = et

            scores = sc_p.tile([128, KCS], F32, tag="sc", name=f"sc{g}")
            encT = []
            pend = []

            def emit_mm1(dd):
                nc.tensor.matmul(
                    scores[:],
                    st["decT"][dd // 4][:, 128 * (dd % 4) : 128 * (dd % 4 + 1)],
                    encT[dd][:],
                    start=(dd == 0),
                    stop=(dd == DT - 1),
                )

            # all transposes first, then the 8 mm1 matmuls back-to-back:
            # bunched matmuls pipeline fill/drain (~230ns pitch) instead of
            # paying the full isolated ~500ns latency each.
            for d in range(DT):
                trt = tr_p.tile([128, 512], F32R, tag="tr", name=f"tr{g}_{d}")
                for n in range(NSUB):
                    nc.tensor.transpose(
                        trt[:, 128 * n : 128 * (n + 1)],
                        et[:, n, 128 * d : 128 * (d + 1)],
                        ident_r[:],
                    )
                eT = encT_p.tile([128, 512], F32R, tag="encT", name=f"eT{g}_{d}")
                psum2sbuf(eT[:], trt[:])
                encT.append(eT)
            for d in range(DT):
                emit_mm1(d)

            if kc == 0:
                nc.vector.reduce_max(
                    st["negmax"][:], scores[:], axis=AX_X, negate=True
                )
                nc.vector.tensor_scalar_add(
                    st["negsh"][:], st["negmax"][:], -25.0
                )
            # shift by -(chunk0max+25) and clamp at +62 before exp: exact
            # softmax (shift-invariant; the clamp only flattens ratios
            # beyond e^62, which are sub-fp32-precision), and neither the
            # exp nor the fp32 accumulations can overflow for any input.
            clamped = scsb_p.tile([128, KCS], F32, tag="scsb", name=f"cl{g}")
            nc.vector.tensor_scalar(
                out=clamped[:],
                in0=scores[:],
                scalar1=st["negsh"][:],
                scalar2=62.0,
                op0=mybir.AluOpType.add,
                op1=mybir.AluOpType.min,
            )
            nc.scalar.activation(
                st["probs"][:, kc * KCS : (kc + 1) * KCS],
                clamped[:],
                EXP,
                bias=0.0,
                scale=1.0,
                accum_out=st["sums"][:, kc : kc + 1],
            )

        def stage_b(g):
            b, kc = divmod(g, KCH)
            st = state[b]
            trt = tr_p.tile([128, 512], F32, tag="tr", name=f"trp{g}")
            for j in range(4):
                t = 4 * kc + j
                nc.tensor.transpose(
                    trt[:, 128 * j : 128 * (j + 1)],
                    st["probs"][:, 128 * t : 128 * (t + 1)],
                    ident[:],
                )
            pT = probsT_p.tile([128, 512], F32R, tag="probsT", name=f"pT{g}")
            psum2sbuf(pT[:], trt[:])
            et = st["enc_sb"][kc]
            for j in range(4):
                t = 4 * kc + j
                for dh in range(DH):
                    nc.tensor.matmul(
                        st["cps"][dh][:],
                        pT[:, 128 * j : 128 * (j + 1)],
                        et[:, j, dh * 512 : (dh + 1) * 512],
                        start=(t == 0),
                        stop=(t == 4 * KCH - 1),
                    )
            if kc == KCH - 1:
                finish_batch(b)

        def finish_batch(b):
            st = state[b]
            denom = stat_p.tile([128, 1], F32, tag="denom", name=f"dn{b}")
            nc.vector.reduce_sum(denom[:], st["sums"][:], axis=AX_X)
            rdenom = stat_p.tile([128, 1], F32, tag="rdenom", name=f"rd{b}")
            nc.vector.reciprocal(rdenom[:], denom[:])
            out_sb = outp_p.tile([128, D], F32, tag="outp", name=f"ou{b}")
            nc.scalar.activation(
                out_sb[:, 0:512], st["cps"][0][:], COPY, bias=0.0, scale=rdenom[:],
            )
            nc.vector.tensor_scalar_mul(
                out_sb[:, 512:1024], st["cps"][1][:], rdenom[:],
            )
            if b == BLOC - 1:
                # input stream is finished; split the last store across
                # SWDGE and HWDGE to halve its exposed latency
                nc.gpsimd.dma_start(out[b][:, 0:512], out_sb[:, 0:512])
                nc.sync.dma_start(out[b][:, 512:1024], out_sb[:, 512:1024])
            else:
                # SWDGE only: never block the Sync input-load queue
                nc.gpsimd.dma_start(out[b], out_sb[:])

        for g in range(NCHUNK):
            stage_a(g)
            if g >= 1:
                stage_b(g - 1)
        stage_b(NCHUNK - 1)

    nc.compile()
    return nc


def kernel(encoder_hiddens: np.ndarray, decoder_hidden: np.ndarray) -> np.ndarray:
    enc = np.ascontiguousarray(np.asarray(encoder_hiddens, dtype=np.float32))
    dec = np.ascontiguousarray(np.asarray(decoder_hidden, dtype=np.float32))
    assert enc.shape == (B, Tk, D) and dec.shape == (B, Tq, D)

    if "nc" not in _CACHE:
        _CACHE["nc"] = _build()
    nc = _CACHE["nc"]

    from concourse.bass_utils import run_bass_kernel_spmd

    in_maps = [
        {
            "enc": enc[c * BLOC : (c + 1) * BLOC],
            "dec": dec[c * BLOC : (c + 1) * BLOC],
        }
        for c in range(CORES)
    ]
    res = run_bass_kernel_spmd(nc, in_maps, core_ids=list(range(CORES)))
    out = np.empty((B, Tq, D), dtype=np.float32)
    for c in range(CORES):
        out[c * BLOC : (c + 1) * BLOC] = res.results[c]["out"]
    return out


# revision 24
# speedup vs baseline: 1.0103x; 1.0103x over previous
"""Trainium2 Bass kernel for cross-attention:
    scores  = dec @ enc^T            [B, Tq, Tk]
    probs   = softmax(scores, -1)
    context = probs @ enc            [B, Tq, D]

Shapes (hardcoded): enc [16, 2048, 1024] f32, dec [16, 128, 1024] f32.
Sharding: data-parallel over batch B across 8 NeuronCores (2 batches/core).

Per-core flow, software-pipelined over 8 global chunks (2 batches x 4
chunks of 512 k-rows):
  stage A(g): DMA chunk; transpose-mode matmuls -> encT [d, k] tiles
              (PSUM -> SBUF copies split DVE/ACT); mm1 (fp32r, N=512)
              accumulates scores [q, 512]; ACT exp streams the chunk
              (softmax shift = chunk-0 row max; exact, shift-invariant)
              with accum_out partial sums.
  stage B(g): probs chunk PE-transposed -> probsT [k, q]; mm2 (fp32r)
              accumulates context from the resident natural enc tiles.
  B(g-1) is emitted after A(g) so the PE never head-of-line blocks on
  the exp -> probsT dependency.
Output stores ride SWDGE (gpsimd) so the Sync DMA queue streams input
chunks back-to-back. A dummy-matmul burst at kernel start (hidden
under the first DMA) flips the PE HAM clock gate to 2.4 GHz early.
fp32r keeps ~13 effective mantissa bits (measured ~1.5e-4).
"""

import sys

sys.path.insert(0, "/opt/trn_rl_repo")

import numpy as np
from contextlib import ExitStack

import concourse.bass as bass
import concourse.tile as tile
from concourse import bacc, mybir
from concourse.masks import make_identity

F32 = mybir.dt.float32
F32R = mybir.dt.float32r
BF16 = mybir.dt.bfloat16
EXP = mybir.ActivationFunctionType.Exp
COPY = mybir.ActivationFunctionType.Copy
AX_X = mybir.AxisListType.X

B, Tk, Tq, D = 16, 2048, 128, 1024
CORES = 8
BLOC = B // CORES          # batches per core
KCH = 4                    # k chunks per batch
KCS = Tk // KCH            # 512 k rows per chunk
NSUB = KCS // 128          # 4 k-subtiles per chunk
DT = D // 128              # 8 d-tiles
DH = D // 512              # 2 output column halves
NCHUNK = BLOC * KCH        # global pipeline length
NWARM = 10                 # dummy matmuls to warm the HAM clock gate

_CACHE = {}


def _build():
    nc = bacc.Bacc("TRN2", debug=False, num_devices=CORES)
    enc = nc.dram_tensor("enc", [BLOC, Tk, D], F32, kind="ExternalInput").ap()
    dec = nc.dram_tensor("dec", [BLOC, Tq, D], F32, kind="ExternalInput").ap()
    out = nc.dram_tensor("out", [BLOC, Tq, D], F32, kind="ExternalOutput").ap()

    with tile.TileContext(nc) as tc, ExitStack() as ctx:
        sb = ctx.enter_context(tc.tile_pool(name="sb", bufs=1))
        enc_p = ctx.enter_context(tc.tile_pool(name="enc", bufs=6))
        encT_p = ctx.enter_context(tc.tile_pool(name="encT", bufs=18))
        dec_p = ctx.enter_context(tc.tile_pool(name="dec", bufs=2))
        decT_p = ctx.enter_context(tc.tile_pool(name="decT", bufs=4))
        probs_p = ctx.enter_context(tc.tile_pool(name="probs", bufs=1))
        probsT_p = ctx.enter_context(tc.tile_pool(name="probsT", bufs=6))
        scsb_p = ctx.enter_context(tc.tile_pool(name="scsb", bufs=3))
        outp_p = ctx.enter_context(tc.tile_pool(name="outp", bufs=2))
        stat_p = ctx.enter_context(tc.tile_pool(name="stat", bufs=4))
        sc_p = ctx.enter_context(tc.tile_pool(name="sc", bufs=2, space="PSUM"))
        tr_p = ctx.enter_context(tc.tile_pool(name="tr", bufs=4, space="PSUM"))
        ctx_p = ctx.enter_context(tc.tile_pool(name="ctx", bufs=2, space="PSUM"))

        # HAM warm-up: independent of the identity-creation chain; only
        # needs the memset. Runs back-to-back while the first enc chunk
        # DMA streams in. fp32 N=128 is 4 cyc/row -> ~0.4us each cold.
        # operand data is irrelevant (the result is never consumed);
        # memset is the fastest producer after the entry barrier.
        junk = sb.tile([128, 128], F32)
        nc.gpsimd.memset(junk[:], 0.0)
        warm = sc_p.tile([128, 512], F32, tag="sc", name="warm")
        for i in range(NWARM):
            nc.tensor.matmul(
                warm[:, 0:128], junk[:], junk[:],
                start=(i == 0), stop=(i == NWARM - 1),
            )

        ident = sb.tile([128, 128], F32)
        ident_r = sb.tile([128, 128], F32R)
        make_identity(nc, ident[:])
        nc.vector.tensor_copy(ident_r[:], ident[:])

        copy_count = [0]

        def psum2sbuf(dst, src):
            if copy_count[0] % 2 == 0:
                nc.vector.tensor_copy(dst, src)
            else:
                nc.scalar.copy(dst, src)
            copy_count[0] += 1

        # per-batch state
        state = {}

        def begin_batch(b):
            st = {}
            st["negmax"] = stat_p.tile([128, 1], F32, tag="negmax", name=f"nm{b}")
            st["negsh"] = stat_p.tile([128, 1], F32, tag="negsh", name=f"ns{b}")
            st["probs"] = probs_p.tile([128, Tk], F32, tag="probs", name=f"pr{b}")
            st["sums"] = stat_p.tile([128, KCH], F32, tag="sums", name=f"sm{b}")
            st["cps"] = [
                ctx_p.tile([128, 512], F32, tag="ctx", name=f"cps{b}_{dh}")
                for dh in range(DH)
            ]
            st["enc_sb"] = {}
            st["encT"] = {}
            # dec load + transposes -> decT [d, q]
            dec_sb = dec_p.tile([128, D], F32R, tag="dec", name=f"dec{b}")
            nc.sync.dma_start(dec_sb[:], dec[b].bitcast(F32R))
            decT = []
            for blk in range(2):
                trt = tr_p.tile([128, 512], F32R, tag="tr", name=f"trd{b}_{blk}")
                for j in range(4):
                    dd = 4 * blk + j
                    nc.tensor.transpose(
                        trt[:, 128 * j : 128 * (j + 1)],
                        dec_sb[:, 128 * dd : 128 * (dd + 1)],
                        ident_r[:],
                    )
                dstT = decT_p.tile([128, 512], F32R, tag="decT", name=f"dT{b}_{blk}")
                psum2sbuf(dstT[:], trt[:])
                decT.append(dstT)
            st["decT"] = decT
            state[b] = st

        def stage_a1(g):
            b, kc = divmod(g, KCH)
            et = enc_p.tile([128, NSUB, D], F32R, tag="enc", name=f"enc{g}")
            nc.sync.dma_start(
                et[:],
                enc[b, kc * KCS : (kc + 1) * KCS, :]
                .rearrange("(n p) d -> p n d", p=128)
                .bitcast(F32R),
            )
            if kc == 0:
                begin_batch(b)
            st = state[b]
            st["enc_sb"][kc] = et
            encT = []
            for d in range(DT):
                trt = tr_p.tile([128, 512], F32R, tag="tr", name=f"tr{g}_{d}")
                for n in range(NSUB):
                    nc.tensor.transpose(
                        trt[:, 128 * n : 128 * (n + 1)],
                        et[:, n, 128 * d : 128 * (d + 1)],
                        ident_r[:],
                    )
                eT = encT_p.tile([128, 512], F32R, tag="encT", name=f"eT{g}_{d}")
                psum2sbuf(eT[:], trt[:])
                encT.append(eT)
            st["encT"][kc] = encT

        def stage_a2(g):
            b, kc = divmod(g, KCH)
            st = state[b]
            encT = st["encT"].pop(kc)
            scores = sc_p.tile([128, KCS], F32, tag="sc", name=f"sc{g}")
            for dd in range(DT):
                nc.tensor.matmul(
                    scores[:],
                    st["decT"][dd // 4][:, 128 * (dd % 4) : 128 * (dd % 4 + 1)],
                    encT[dd][:],
                    start=(dd == 0),
                    stop=(dd == DT - 1),
                )
            if kc == 0:
                nc.vector.reduce_max(
                    st["negmax"][:], scores[:], axis=AX_X, negate=True
                )
                nc.vector.tensor_scalar_add(
                    st["negsh"][:], st["negmax"][:], -25.0
                )
            # shift by -(chunk0max+25) and clamp at +62 before exp: exact
            # softmax (shift-invariant; the clamp only flattens ratios
            # beyond e^62, which are sub-fp32-precision), and neither the
            # exp nor the fp32 accumulations can overflow for any input.
            clamped = scsb_p.tile([128, KCS], F32, tag="scsb", name=f"cl{g}")
            nc.vector.tensor_scalar(
                out=clamped[:],
                in0=scores[:],
                scalar1=st["negsh"][:],
                scalar2=62.0,
                op0=mybir.AluOpType.add,
                op1=mybir.AluOpType.min,
            )
            nc.scalar.activation(
                st["probs"][:, kc * KCS : (kc + 1) * KCS],
                clamped[:],
                EXP,
                bias=0.0,
                scale=1.0,
                accum_out=st["sums"][:, kc : kc + 1],
            )

        def stage_b(g):
            b, kc = divmod(g, KCH)
            st = state[b]
            trt = tr_p.tile([128, 512], F32, tag="tr", name=f"trp{g}")
            for j in range(4):
                t = 4 * kc + j
                nc.tensor.transpose(
                    trt[:, 128 * j : 128 * (j + 1)],
                    st["probs"][:, 128 * t : 128 * (t + 1)],
                    ident[:],
                )
            pT = probsT_p.tile([128, 512], F32R, tag="probsT", name=f"pT{g}")
            psum2sbuf(pT[:], trt[:])
            et = st["enc_sb"][kc]
            for j in range(4):
                t = 4 * kc + j
                for dh in range(DH):
                    nc.tensor.matmul(
                        st["cps"][dh][:],
                        pT[:, 128 * j : 128 * (j + 1)],
                        et[:, j, dh * 512 : (dh + 1) * 512],
                        start=(t == 0),
                        stop=(t == 4 * KCH - 1),
                    )
            if kc == KCH - 1:
                finish_batch(b)

        def finish_batch(b):
            st = state[b]
            denom = stat_p.tile([128, 1], F32, tag="denom", name=f"dn{b}")
            nc.vector.reduce_sum(denom[:], st["sums"][:], axis=AX_X)
            rdenom = stat_p.tile([128, 1], F32, tag="rdenom", name=f"rd{b}")
            nc.vector.reciprocal(rdenom[:], denom[:])
            out_sb = outp_p.tile([128, D], F32, tag="outp", name=f"ou{b}")
            nc.scalar.activation(
                out_sb[:, 0:512], st["cps"][0][:], COPY, bias=0.0, scale=rdenom[:],
            )
            nc.vector.tensor_scalar_mul(
                out_sb[:, 512:1024], st["cps"][1][:], rdenom[:],
            )
            if b == BLOC - 1:
                # input stream is finished; split the last store across
                # SWDGE and HWDGE to halve its exposed latency
                nc.gpsimd.dma_start(out[b][:, 0:512], out_sb[:, 0:512])
                nc.sync.dma_start(out[b][:, 512:1024], out_sb[:, 512:1024])
            else:
                # SWDGE only: never block the Sync input-load queue
                nc.gpsimd.dma_start(out[b], out_sb[:])

        for g in range(NCHUNK + 2):
            if g < NCHUNK:
                stage_a1(g)
            if 1 <= g + 0 and g - 1 >= 0 and g - 1 < NCHUNK:
                stage_a2(g - 1)
            if g - 2 >= 0:
                stage_b(g - 2)

    nc.compile()
    return nc


def kernel(encoder_hiddens: np.ndarray, decoder_hidden: np.ndarray) -> np.ndarray:
    enc = np.ascontiguousarray(np.asarray(encoder_hiddens, dtype=np.float32))
    dec = np.ascontiguousarray(np.asarray(decoder_hidden, dtype=np.float32))
    assert enc.shape == (B, Tk, D) and dec.shape == (B, Tq, D)

    if "nc" not in _CACHE:
        _CACHE["nc"] = _build()
    nc = _CACHE["nc"]

    from concourse.bass_utils import run_bass_kernel_spmd

    in_maps = [
        {
            "enc": enc[c * BLOC : (c + 1) * BLOC],
            "dec": dec[c * BLOC : (c + 1) * BLOC],
        }
        for c in range(CORES)
    ]
    res = run_bass_kernel_spmd(nc, in_maps, core_ids=list(range(CORES)))
    out = np.empty((B, Tq, D), dtype=np.float32)
    for c in range(CORES):
        out[c * BLOC : (c + 1) * BLOC] = res.results[c]["out"]
    return out
